# revision 17
# baseline (speedup 1.0000x reference)
"""Multi-head attention Trainium2 kernel (8 NeuronCores).

Sharding: 8 cores = 2 batches x 4 head-groups (4 heads each, tensor-parallel
over heads per the hint, data-parallel over batch).

Per-core device program (all layouts "transposed": contraction dim on
partitions):
  QT = (Wq_g * s) @ q_b^T + bq_g*s     [256, 2048]   (s = 1/sqrt(64) folded)
  KT =  Wk_g      @ k_b^T + bk_g       [256, 2048]
  VT =  Wv_g      @ v_b^T              [256, 2048] -> PE-transpose -> V' [j, e]
       (V' has a ones column per head -> PV matmul also accumulates sumexp)
  scores^T[j,i] = KT_h^T-chunk.T @ QT_h   (K=64, heads pair-packed in PE rows)
  P^T = exp(scores^T)                  (no max subtraction; scores ~ N(0,1))
  acc_h[65, i]  = [V_h | 1]^T @ P^T    (row 64 = sumexp)
  att_h = acc_h[0:64] * (1/sumexp)     (broadcast via K=1 matmul with ones)
  outT[o, i]   += Wo_h-chunk^T @ att_h (accumulate over 4 heads)
Host: un-transpose, sum the 4 head-group partials per batch, add
bo + Wo @ bv (the V-bias commutes to a constant through softmax rows
summing to 1).
"""

import sys

if "/opt/trn_rl_repo" not in sys.path:
    sys.path.insert(0, "/opt/trn_rl_repo")

import numpy as np

import concourse.bass as bass
import concourse.tile as tile
from concourse import bacc
from concourse import mybir
from concourse.bass_utils import run_bass_kernel_spmd
from concourse.masks import make_identity

F32 = mybir.dt.float32
F32R = mybir.dt.float32r
F16 = mybir.dt.float16

B, S, D, H = 2, 2048, 1024, 16
HD = 64          # head dim
HPC = 4          # heads per core
DC = HPC * HD    # 256 output dims per core
P = 128
CT = D // P      # 8 contraction tiles (projection K)
NIC = 4          # i-chunks
ICW = S // NIC   # 512
NJT = S // P     # 16 j-tiles
NOT = D // P     # 8 output o-tiles

EXP = mybir.ActivationFunctionType.Exp
LN = mybir.ActivationFunctionType.Ln

_CACHED_NC = None


def r(ap):
    return ap  # operands are declared float32r natively


def build_nc():
    nc = bacc.Bacc("TRN2", target_bir_lowering=False, debug=False)

    xq = nc.dram_tensor("xq", [P, CT, S], F16, kind="ExternalInput").ap()
    xk = nc.dram_tensor("xk", [P, CT, S], F16, kind="ExternalInput").ap()
    xv = nc.dram_tensor("xv", [P, CT, S], F16, kind="ExternalInput").ap()
    wq = nc.dram_tensor("wq", [P, CT, DC], F16, kind="ExternalInput").ap()
    wk = nc.dram_tensor("wk", [P, CT, DC], F16, kind="ExternalInput").ap()
    wv = nc.dram_tensor("wv", [P, CT, DC], F16, kind="ExternalInput").ap()
    wo = nc.dram_tensor("wo", [HD, HPC, D], F16, kind="ExternalInput").ap()
    bq = nc.dram_tensor("bq", [P, 2], F32, kind="ExternalInput").ap()
    bk = nc.dram_tensor("bk", [P, 2], F32, kind="ExternalInput").ap()
    onec = nc.dram_tensor("onec", [P, HD], F16, kind="ExternalInput").ap()
    outT = nc.dram_tensor("outT", [NOT, P, S], F32, kind="ExternalOutput").ap()

    with tile.TileContext(nc) as tc:
        with tc.tile_pool(name="w", bufs=1) as wpool, \
             tc.tile_pool(name="persist", bufs=1) as persist:
            w_q = wpool.tile([P, CT, DC], F16)
            w_k = wpool.tile([P, CT, DC], F16)
            w_v = wpool.tile([P, CT, DC], F16)
            w_oh = wpool.tile([HD, HPC, D], F16)
            bq_sb = wpool.tile([P, 2], F32)
            bk_sb = wpool.tile([P, 2], F32)
            ident = wpool.tile([P, P], F16)
            ones_sb = wpool.tile([P, HD], F16)

            QT = persist.tile([P, 2, S], F16)    # [p, dtile, i]
            KT = persist.tile([P, 2, S], F16)
            Vp = persist.tile([P, NJT, HPC, HD + 1], F16)  # V' + ones col
            att4 = persist.tile([HD, HPC, S], F16)         # per-head, base 0

            # ---------------- projections ----------------
            with tc.tile_pool(name="xs", bufs=4) as xpool, \
                 tc.tile_pool(name="pp", bufs=8, space="PSUM") as ppool, \
                 tc.tile_pool(name="vt", bufs=4) as vtpool:
                for xin, wsb, wdram, dst, bias in (
                    (xq, w_q, wq, QT, bq_sb),
                    (xk, w_k, wk, KT, bk_sb),
                    (xv, w_v, wv, None, None),
                ):
                    psums = [[ppool.tile([P, ICW], F32, tag="proj", name=f"ps_{t}_{i}")
                              for i in range(NIC)] for t in range(2)]
                    for ct in range(CT):
                        if wdram is not None:
                            nc.sync.dma_start(wsb[:, ct, :], wdram[:, ct, :])
                        x_sb = xpool.tile([P, S], F16, tag="xstream")
                        nc.sync.dma_start(x_sb[:], xin[:, ct, :])
                        if ct == 2 and wdram is wq:
                            # deferred const loads: queue behind the first
                            # chunk so the first matmul starts ASAP
                            nc.sync.dma_start(w_oh[:], wo)
                            nc.sync.dma_start(bq_sb[:], bq)
                            nc.sync.dma_start(bk_sb[:], bk)
                            nc.sync.dma_start(ones_sb[:], onec)
                            make_identity(nc, ident[:])
                            nc.sync.dma_start(
                                Vp[:, :, :, HD:HD + 1],
                                onec.rearrange(
                                    "p (j h) -> p j h", j=NJT)[:, :, :, None])
                        for t in range(2):
                            for ic in range(NIC):
                                nc.tensor.matmul(
                                    psums[t][ic][:, :],
                                    lhsT=r(wsb[:, ct, t * P:(t + 1) * P]),
                                    rhs=r(x_sb[:, ic * ICW:(ic + 1) * ICW]),
                                    start=(ct == 0), stop=(ct == CT - 1),
                                )
                    for t in range(2):
                        for ic in range(NIC):
                            isl = slice(ic * ICW, (ic + 1) * ICW)
                            if dst is not None:
                                nc.vector.tensor_scalar_add(
                                    dst[:, t, isl], psums[t][ic][:, :],
                                    bias[:, t:t + 1],
                                )
                            else:
                                # V^T chunk -> transpose -> V' natural layout
                                vtmp = vtpool.tile([P, ICW], F16, tag="vt")
                                nc.vector.tensor_copy(vtmp[:], psums[t][ic][:, :])
                                for jj in range(ICW // P):
                                    jt = ic * (ICW // P) + jj
                                    tp = ppool.tile([P, P], F16, tag="proj")
                                    nc.tensor.transpose(
                                        tp[:, :],
                                        vtmp[:, jj * P:(jj + 1) * P],
                                        ident[:],
                                    )
                                    nc.vector.tensor_copy(
                                        Vp[:, jt, 2 * t:2 * t + 2, 0:HD],
                                        tp[:, :].rearrange(
                                            "p (h e) -> p h e", h=2),
                                    )

            # ---------------- attention ----------------
            with tc.tile_pool(name="sc", bufs=2, space="PSUM") as spool, \
                 tc.tile_pool(name="acc", bufs=4, space="PSUM") as apool, \
                 tc.tile_pool(name="pt", bufs=6) as ptpool, \
                 tc.tile_pool(name="nrm", bufs=2) as npool, \
                 tc.tile_pool(name="oev", bufs=4) as oevpool:
                for ic in range(NIC):
                    isl = slice(ic * ICW, (ic + 1) * ICW)
                    accs = [apool.tile([HD + 1, ICW], F32, tag="acc", name=f"acc_{h}")
                            for h in range(HPC)]
                    for jt in range(NJT):
                        for hp in range(2):
                            sc = spool.tile([P, 2 * ICW], F32, tag="sc")
                            for hl in range(2):
                                psl = slice(64 * hl, 64 * (hl + 1))
                                nc.tensor.matmul(
                                    sc[:, hl * ICW:(hl + 1) * ICW],
                                    lhsT=r(KT[psl, hp, jt * P:(jt + 1) * P]),
                                    rhs=r(QT[psl, hp, isl]),
                                    start=True, stop=True,
                                )
                            pt = ptpool.tile([P, 2 * ICW], F16, tag="pt")
                            nc.scalar.activation(pt[:], sc[:], EXP)
                            for hl in range(2):
                                h = 2 * hp + hl
                                nc.tensor.matmul(
                                    accs[h][:, :],
                                    lhsT=r(Vp[:, jt, h, :]),
                                    rhs=r(pt[:, hl * ICW:(hl + 1) * ICW]),
                                    start=(jt == 0), stop=(jt == NJT - 1),
                                )
                    au = npool.tile([HD + 1, HPC, ICW], F32, tag="aun")
                    for h in range(HPC):
                        nc.vector.tensor_copy(au[:, h, :], accs[h][:, :])
                    # 1/sumexp = exp(-ln(sumexp)) on ACT, all 4 heads at once
                    nc.scalar.activation(
                        au[HD:HD + 1, :, :], au[HD:HD + 1, :, :], LN)
                    rec16 = npool.tile([HD + 1, HPC, ICW], F16, tag="rec16")
                    nc.scalar.activation(
                        rec16[HD:HD + 1, :, :], au[HD:HD + 1, :, :], EXP,
                        scale=-1.0)
                    for h in range(HPC):
                        bcp = apool.tile([HD, ICW], F32, tag="acc")
                        nc.tensor.matmul(
                            bcp[:, :],
                            lhsT=ones_sb[HD:HD + 1, :],
                            rhs=rec16[HD:HD + 1, h, :],
                            start=True, stop=True,
                        )
                        nc.vector.tensor_mul(
                            att4[:, h, isl], au[0:HD, h, :], bcp[:, :])

                    # ---------------- output projection ----------------
                    for ot in range(NOT):
                        po = apool.tile([P, ICW], F32, tag="acc")
                        for h in range(HPC):
                            nc.tensor.matmul(
                                po[:, :],
                                lhsT=r(w_oh[:, h, ot * P:(ot + 1) * P]),
                                rhs=r(att4[:, h, isl]),
                                start=(h == 0), stop=(h == HPC - 1),
                            )
                        osb = oevpool.tile([P, ICW], F32, tag="oev")
                        nc.vector.tensor_copy(osb[:], po[:, :])
                        nc.sync.dma_start(outT[ot, :, isl], osb[:])
    nc.compile()
    return nc


def get_nc():
    global _CACHED_NC
    if _CACHED_NC is None:
        _CACHED_NC = build_nc()
    return _CACHED_NC


def round_fp32r(a):
    u = np.ascontiguousarray(a, np.float32).view(np.uint32)
    low = u & np.uint32(0xFFF)
    base = u & ~np.uint32(0xFFF)
    odd = (base >> np.uint32(12)) & np.uint32(1)
    up = (low > 0x800) | ((low == 0x800) & (odd == 1))
    return (base + np.where(up, np.uint32(0x1000), np.uint32(0))).view(np.float32)


def _prep_x(x):
    # [S, D] -> [P, CT, S] : dev[p, ct, i] = x[i, ct*128+p]
    return np.ascontiguousarray(
        x.reshape(S, CT, P).transpose(2, 1, 0)).astype(np.float16)


def _prep_w(w, scale=1.0):
    # W slice [DC, D] -> [P, CT, DC] : dev[p, ct, d] = W[d, ct*128+p]*scale
    wT = (w.T * scale).astype(np.float32)  # [D, DC]
    return np.ascontiguousarray(
        wT.reshape(CT, P, DC).transpose(1, 0, 2)).astype(np.float16)


def make_in_maps(q, k, v, Wq, bq, Wk, bk, Wv, bv, Wo, bo):
    scale = 1.0 / np.sqrt(HD)
    xs = {}
    for b in range(B):
        xs[b] = (_prep_x(q[b]), _prep_x(k[b]), _prep_x(v[b]))
    in_maps = []
    for core in range(8):
        b, g = divmod(core, 4)
        gs = slice(g * DC, (g + 1) * DC)
        woT = Wo[:, gs].T  # [DC, D]
        in_maps.append({
            "xq": xs[b][0], "xk": xs[b][1], "xv": xs[b][2],
            "wq": _prep_w(Wq[gs], scale),
            "wk": _prep_w(Wk[gs]),
            "wv": _prep_w(Wv[gs]),
            "wo": np.ascontiguousarray(
                woT.reshape(HPC, HD, D).transpose(1, 0, 2)).astype(np.float16),
            "bq": np.ascontiguousarray(
                (bq[gs] * scale).reshape(2, P).T).astype(np.float32),
            "bk": np.ascontiguousarray(
                bk[gs].reshape(2, P).T).astype(np.float32),
            "onec": np.ones((P, HD), np.float16),
        })
    return in_maps


def kernel(q, k, v, Wq, bq, Wk, bk, Wv, bv, Wo, bo, _results_hook=None):
    q = np.asarray(q, np.float32)
    k = np.asarray(k, np.float32)
    v = np.asarray(v, np.float32)
    Wq = np.asarray(Wq, np.float32)
    Wk = np.asarray(Wk, np.float32)
    Wv = np.asarray(Wv, np.float32)
    Wo = np.asarray(Wo, np.float32)
    bq = np.asarray(bq, np.float32)
    bk = np.asarray(bk, np.float32)
    bv = np.asarray(bv, np.float32)
    bo = np.asarray(bo, np.float32)

    nc = get_nc()
    in_maps = make_in_maps(q, k, v, Wq, bq, Wk, bk, Wv, bv, Wo, bo)
    res = run_bass_kernel_spmd(nc, in_maps, core_ids=list(range(8)))
    if _results_hook is not None:
        _results_hook(res)

    const = bo + Wo @ bv  # V-bias folds to a constant through softmax
    out = np.zeros((B, S, D), np.float32)
    for core in range(8):
        b = core // 4
        pT = res.results[core]["outT"]  # [NOT, P, S]
        out[b] += pT.transpose(2, 0, 1).reshape(S, D).astype(np.float32)
    out += const[None, None, :]
    return out


# revision 19
# speedup vs baseline: 1.1024x; 1.1024x over previous
"""Multi-head attention Trainium2 kernel (8 NeuronCores).

Sharding: 8 cores = 2 batches x 4 head-groups (4 heads each, tensor-parallel
over heads per the hint, data-parallel over batch).

Per-core device program (all layouts "transposed": contraction dim on
partitions; all matmuls fp16 with fp32 PSUM accumulation):
  QT = (Wq_g * s) @ q_b^T + bq_g*s     [256, 2048]   (s = 1/sqrt(64) folded)
  KT =  Wk_g      @ k_b^T + bk_g       [256, 2048]
  VT =  Wv_g      @ v_b^T              [256, 2048] -> PE-transpose -> V' [j, e]
       (V' has a ones column per head -> PV matmul also accumulates sumexp)
  scores^T[j,i] = KT_h-chunk.T @ QT_h  (K=64, head pairs row-packed in PE)
  P^T = exp(scores^T)                  (no max subtraction; scores ~ N(0,1))
  acc_h[65, i]  = [V_h | 1].T @ P^T    (row 64 = sumexp)
  1/sumexp      = exp(-ln(sumexp))     (ACT; avoids slow DVE reciprocal)
  att_h = acc_h[0:64] * bcast(1/sumexp)  (broadcast via K=1 fp16 matmul)
  outT[o, i]   += Wo_h-chunk.T @ att_h (accumulate over 4 heads)
The norm + output-projection of chunk ic is traced after the attention loop
of chunk ic+1 so the in-order PE stream never stalls on the norm chain.
Host: un-transpose, sum the 4 head-group partials per batch, add
bo + Wo @ bv (the V-bias commutes to a constant through softmax rows
summing to 1).
"""

import sys

if "/opt/trn_rl_repo" not in sys.path:
    sys.path.insert(0, "/opt/trn_rl_repo")

import numpy as np

import concourse.bass as bass  # noqa: F401
import concourse.tile as tile
from concourse import bacc, mybir
from concourse.bass_utils import run_bass_kernel_spmd
from concourse.masks import make_identity

F32 = mybir.dt.float32
F16 = mybir.dt.float16

B, S, D, H = 2, 2048, 1024, 16
HD = 64          # head dim
HPC = 4          # heads per core
DC = HPC * HD    # 256 output dims per core
P = 128
CT = D // P      # 8 contraction tiles (projection K)
NIC = 4          # i chunks
ICW = S // NIC   # 512
NJT = S // P     # 16 j tiles
NOT = D // P     # 8 output o-tiles

EXP = mybir.ActivationFunctionType.Exp
LN = mybir.ActivationFunctionType.Ln

_CACHED_NC = None


def build_nc():
    nc = bacc.Bacc("TRN2", target_bir_lowering=False, debug=False)

    xq = nc.dram_tensor("xq", [P, CT, S], F16, kind="ExternalInput").ap()
    xk = nc.dram_tensor("xk", [P, CT, S], F16, kind="ExternalInput").ap()
    xv = nc.dram_tensor("xv", [P, CT, S], F16, kind="ExternalInput").ap()
    wq = nc.dram_tensor("wq", [P, CT, DC], F16, kind="ExternalInput").ap()
    wk = nc.dram_tensor("wk", [P, CT, DC], F16, kind="ExternalInput").ap()
    wv = nc.dram_tensor("wv", [P, CT, DC], F16, kind="ExternalInput").ap()
    wo = nc.dram_tensor("wo", [HD, HPC, D], F16, kind="ExternalInput").ap()
    bq = nc.dram_tensor("bq", [P, 2], F32, kind="ExternalInput").ap()
    bk = nc.dram_tensor("bk", [P, 2], F32, kind="ExternalInput").ap()
    onec = nc.dram_tensor("onec", [P, HD], F16, kind="ExternalInput").ap()
    outT = nc.dram_tensor("outT", [NOT, P, S], F32, kind="ExternalOutput").ap()

    with tile.TileContext(nc) as tc:
        with tc.tile_pool(name="w", bufs=1) as wpool, \
             tc.tile_pool(name="persist", bufs=1) as persist:
            w_q = wpool.tile([P, CT, DC], F16)
            w_k = wpool.tile([P, CT, DC], F16)
            w_v = wpool.tile([P, CT, DC], F16)
            w_oh = wpool.tile([HD, HPC, D], F16)
            bq_sb = wpool.tile([P, 2], F32)
            bk_sb = wpool.tile([P, 2], F32)
            ident = wpool.tile([P, P], F16)
            ones_sb = wpool.tile([P, HD], F16)

            QT = persist.tile([P, 2, S], F16)    # [p, dtile, i]
            KT = persist.tile([P, 2, S], F16)
            Vp = persist.tile([P, NJT, HPC, HD + 1], F16)  # V' + ones col
            att4 = persist.tile([HD, HPC, S], F16)         # per-head, base 0

            # ---------------- projections ----------------
            with tc.tile_pool(name="xs", bufs=8) as xpool, \
                 tc.tile_pool(name="pp", bufs=8, space="PSUM") as ppool, \
                 tc.tile_pool(name="vt", bufs=4) as vtpool:
                for xin, wsb, wdram, dst, bias in (
                    (xq, w_q, wq, QT, bq_sb),
                    (xk, w_k, wk, KT, bk_sb),
                    (xv, w_v, wv, None, None),
                ):
                    psums = [[ppool.tile([P, ICW], F32, tag="proj",
                                         name=f"ps_{t}_{i}")
                              for i in range(NIC)] for t in range(2)]
                    for ct in range(CT):
                        nc.sync.dma_start(wsb[:, ct, :], wdram[:, ct, :])
                        # split x chunk into halves so matmuls start sooner
                        xh = [xpool.tile([P, S // 2], F16, tag="xstream",
                                         name=f"xh_{ct}_{half}")
                              for half in range(2)]
                        for half in range(2):
                            nc.sync.dma_start(
                                xh[half][:],
                                xin[:, ct,
                                    half * (S // 2):(half + 1) * (S // 2)])
                        if ct == 2 and wdram is wq:
                            # deferred const loads: queue behind the first
                            # chunks so the first matmuls start ASAP
                            nc.sync.dma_start(w_oh[:], wo)
                            nc.sync.dma_start(bq_sb[:], bq)
                            nc.sync.dma_start(bk_sb[:], bk)
                            nc.sync.dma_start(ones_sb[:], onec)
                            make_identity(nc, ident[:])
                            nc.sync.dma_start(
                                Vp[:, :, :, HD:HD + 1],
                                onec.rearrange(
                                    "p (j h) -> p j h", j=NJT)[:, :, :, None])
                        for t in range(2):
                            for ic in range(NIC):
                                half, sub = divmod(ic, 2)
                                nc.tensor.matmul(
                                    psums[t][ic][:, :],
                                    lhsT=wsb[:, ct, t * P:(t + 1) * P],
                                    rhs=xh[half][:, sub * ICW:(sub + 1) * ICW],
                                    start=(ct == 0), stop=(ct == CT - 1),
                                )
                    for t in range(2):
                        for ic in range(NIC):
                            isl = slice(ic * ICW, (ic + 1) * ICW)
                            if dst is not None:
                                nc.vector.tensor_scalar_add(
                                    dst[:, t, isl], psums[t][ic][:, :],
                                    bias[:, t:t + 1],
                                )
                            else:
                                # V^T chunk -> transpose -> V' natural layout
                                vtmp = vtpool.tile([P, ICW], F16, tag="vt")
                                nc.vector.tensor_copy(
                                    vtmp[:], psums[t][ic][:, :])
                                for jj in range(ICW // P):
                                    jt = ic * (ICW // P) + jj
                                    tp = ppool.tile([P, P], F16, tag="proj",
                                                    name=f"tp_{t}_{ic}_{jj}")
                                    nc.tensor.transpose(
                                        tp[:, :],
                                        vtmp[:, jj * P:(jj + 1) * P],
                                        ident[:],
                                    )
                                    nc.vector.tensor_copy(
                                        Vp[:, jt, 2 * t:2 * t + 2, 0:HD],
                                        tp[:, :].rearrange(
                                            "p (h e) -> p h e", h=2),
                                    )

            # ---------------- attention + output projection ----------------
            with tc.tile_pool(name="sc", bufs=2, space="PSUM") as spool, \
                 tc.tile_pool(name="acc", bufs=4, space="PSUM") as apool, \
                 tc.tile_pool(name="pt", bufs=6) as ptpool, \
                 tc.tile_pool(name="nrm", bufs=2) as npool, \
                 tc.tile_pool(name="oev", bufs=4) as oevpool:

                def jt_loop(ic):
                    isl = slice(ic * ICW, (ic + 1) * ICW)
                    accs = [apool.tile([HD + 1, ICW], F32, tag="acc",
                                       name=f"acc_{ic}_{h}")
                            for h in range(HPC)]
                    for jt in range(NJT):
                        for hp in range(2):
                            sc = spool.tile([P, 2 * ICW], F32, tag="sc",
                                            name=f"sc_{ic}_{jt}_{hp}")
                            for hl in range(2):
                                psl = slice(64 * hl, 64 * (hl + 1))
                                nc.tensor.matmul(
                                    sc[:, hl * ICW:(hl + 1) * ICW],
                                    lhsT=KT[psl, hp, jt * P:(jt + 1) * P],
                                    rhs=QT[psl, hp, isl],
                                    start=True, stop=True,
                                )
                            pt = ptpool.tile([P, 2 * ICW], F16, tag="pt",
                                             name=f"pt_{ic}_{jt}_{hp}")
                            nc.scalar.activation(pt[:], sc[:], EXP)
                            for hl in range(2):
                                h = 2 * hp + hl
                                nc.tensor.matmul(
                                    accs[h][:, :],
                                    lhsT=Vp[:, jt, h, :],
                                    rhs=pt[:, hl * ICW:(hl + 1) * ICW],
                                    start=(jt == 0), stop=(jt == NJT - 1),
                                )
                    return accs

                def norm_a(ic, accs):
                    au = npool.tile([HD + 1, HPC, ICW], F32, tag="aun",
                                    name=f"au_{ic}")
                    for h in range(HPC):
                        nc.vector.tensor_copy(au[:, h, :], accs[h][:, :])
                    # 1/sumexp = exp(-ln(sumexp)) on ACT, all heads at once
                    nc.scalar.activation(
                        au[HD:HD + 1, :, :], au[HD:HD + 1, :, :], LN)
                    rec16 = npool.tile([HD + 1, HPC, ICW], F16, tag="rec16",
                                       name=f"rec16_{ic}")
                    nc.scalar.activation(
                        rec16[HD:HD + 1, :, :], au[HD:HD + 1, :, :], EXP,
                        scale=-1.0)
                    return au, rec16

                def norm_b(ic, au, rec16):
                    isl = slice(ic * ICW, (ic + 1) * ICW)
                    for h in range(HPC):
                        bcp = apool.tile([HD, ICW], F32, tag="acc",
                                         name=f"bcp_{ic}_{h}")
                        nc.tensor.matmul(
                            bcp[:, :],
                            lhsT=ones_sb[HD:HD + 1, :],
                            rhs=rec16[HD:HD + 1, h, :],
                            start=True, stop=True,
                        )
                        nc.vector.tensor_mul(
                            att4[:, h, isl], au[0:HD, h, :], bcp[:, :])
                    for ot in range(NOT):
                        po = apool.tile([P, ICW], F32, tag="acc",
                                        name=f"po_{ic}_{ot}")
                        for h in range(HPC):
                            nc.tensor.matmul(
                                po[:, :],
                                lhsT=w_oh[:, h, ot * P:(ot + 1) * P],
                                rhs=att4[:, h, isl],
                                start=(h == 0), stop=(h == HPC - 1),
                            )
                        osb = oevpool.tile([P, ICW], F32, tag="oev",
                                           name=f"osb_{ic}_{ot}")
                        nc.vector.tensor_copy(osb[:], po[:, :])
                        nc.sync.dma_start(outT[ot, :, isl], osb[:])

                pend = None
                for ic in range(NIC):
                    accs = jt_loop(ic)
                    au, rec16 = norm_a(ic, accs)
                    if pend is not None:
                        norm_b(*pend)
                    pend = (ic, au, rec16)
                norm_b(*pend)
    nc.compile()
    return nc


def get_nc():
    global _CACHED_NC
    if _CACHED_NC is None:
        _CACHED_NC = build_nc()
    return _CACHED_NC


def _prep_x(x):
    # [S, D] -> [P, CT, S] : dev[p, ct, i] = x[i, ct*128+p]
    return np.ascontiguousarray(
        x.reshape(S, CT, P).transpose(2, 1, 0)).astype(np.float16)


def _prep_w(w, scale=1.0):
    # W slice [DC, D] -> [P, CT, DC] : dev[p, ct, d] = W[d, ct*128+p]*scale
    wT = (w.T * scale).astype(np.float32)  # [D, DC]
    return np.ascontiguousarray(
        wT.reshape(CT, P, DC).transpose(1, 0, 2)).astype(np.float16)


def make_in_maps(q, k, v, Wq, bq, Wk, bk, Wv, bv, Wo, bo):
    scale = 1.0 / np.sqrt(HD)
    xs = {}
    for b in range(B):
        xs[b] = (_prep_x(q[b]), _prep_x(k[b]), _prep_x(v[b]))
    in_maps = []
    for core in range(8):
        b, g = divmod(core, 4)
        gs = slice(g * DC, (g + 1) * DC)
        woT = Wo[:, gs].T  # [DC, D]
        in_maps.append({
            "xq": xs[b][0], "xk": xs[b][1], "xv": xs[b][2],
            "wq": _prep_w(Wq[gs], scale),
            "wk": _prep_w(Wk[gs]),
            "wv": _prep_w(Wv[gs]),
            "wo": np.ascontiguousarray(
                woT.reshape(HPC, HD, D).transpose(1, 0, 2)).astype(np.float16),
            "bq": np.ascontiguousarray(
                (bq[gs] * scale).reshape(2, P).T).astype(np.float32),
            "bk": np.ascontiguousarray(
                bk[gs].reshape(2, P).T).astype(np.float32),
            "onec": np.ones((P, HD), np.float16),
        })
    return in_maps


def kernel(q, k, v, Wq, bq, Wk, bk, Wv, bv, Wo, bo, _results_hook=None):
    q = np.asarray(q, np.float32)
    k = np.asarray(k, np.float32)
    v = np.asarray(v, np.float32)
    Wq = np.asarray(Wq, np.float32)
    Wk = np.asarray(Wk, np.float32)
    Wv = np.asarray(Wv, np.float32)
    Wo = np.asarray(Wo, np.float32)
    bq = np.asarray(bq, np.float32)
    bk = np.asarray(bk, np.float32)
    bv = np.asarray(bv, np.float32)
    bo = np.asarray(bo, np.float32)

    nc = get_nc()
    in_maps = make_in_maps(q, k, v, Wq, bq, Wk, bk, Wv, bv, Wo, bo)
    res = run_bass_kernel_spmd(nc, in_maps, core_ids=list(range(8)))
    if _results_hook is not None:
        _results_hook(res)

    const = bo + Wo @ bv  # V-bias folds to a constant through softmax
    out = np.zeros((B, S, D), np.float32)
    for core in range(8):
        b = core // 4
        pT = res.results[core]["outT"]  # [NOT, P, S]
        out[b] += pT.transpose(2, 0, 1).reshape(S, D).astype(np.float32)
    out += const[None, None, :]
    return out


# revision 20
# speedup vs baseline: 1.1336x; 1.0283x over previous
"""Multi-head attention Trainium2 kernel (8 NeuronCores).

Sharding: 8 cores = 2 batches x 4 head-groups (4 heads each, tensor-parallel
over heads per the hint, data-parallel over batch).

Per-core device program (all layouts "transposed": contraction dim on
partitions; all matmuls fp16 with fp32 PSUM accumulation):
  QT = (Wq_g * s) @ q_b^T + bq_g*s     [256, 2048]   (s = 1/sqrt(64) folded)
  KT =  Wk_g      @ k_b^T + bk_g       [256, 2048]
  VT =  Wv_g      @ v_b^T              [256, 2048] -> PE-transpose -> V' [j, e]
       (V' has a ones column per head -> PV matmul also accumulates sumexp)
  scores^T[j,i] = KT_h-chunk.T @ QT_h  (K=64, head pairs row-packed in PE)
  P^T = exp(scores^T)                  (no max subtraction; scores ~ N(0,1))
  acc_h[65, i]  = [V_h | 1].T @ P^T    (row 64 = sumexp)
  1/sumexp      = exp(-ln(sumexp))     (ACT; avoids slow DVE reciprocal)
  att_h = acc_h[0:64] * bcast(1/sumexp)  (broadcast via K=1 fp16 matmul)
  outT[o, i]   += Wo_h-chunk.T @ att_h (accumulate over 4 heads)
The norm + output-projection of chunk ic is traced after the attention loop
of chunk ic+1 so the in-order PE stream never stalls on the norm chain.
Host: un-transpose, sum the 4 head-group partials per batch, add
bo + Wo @ bv (the V-bias commutes to a constant through softmax rows
summing to 1).
"""

import sys

if "/opt/trn_rl_repo" not in sys.path:
    sys.path.insert(0, "/opt/trn_rl_repo")

import numpy as np

import concourse.bass as bass  # noqa: F401
import concourse.tile as tile
from concourse import bacc, mybir
from concourse.bass_utils import run_bass_kernel_spmd
from concourse.masks import make_identity

F32 = mybir.dt.float32
F16 = mybir.dt.float16

B, S, D, H = 2, 2048, 1024, 16
HD = 64          # head dim
HPC = 4          # heads per core
DC = HPC * HD    # 256 output dims per core
P = 128
CT = D // P      # 8 contraction tiles (projection K)
NIC = 4          # i chunks
ICW = S // NIC   # 512
NJT = S // P     # 16 j tiles
NOT = D // P     # 8 output o-tiles

EXP = mybir.ActivationFunctionType.Exp
LN = mybir.ActivationFunctionType.Ln

_CACHED_NC = None


def _steer_act_tables():
    """Make natural_log_exp_and_others the only set serving Exp/Ln so the
    table chooser emits a single ACT_TABLE_LOAD instead of thrashing
    between the exp and ln sets every chunk. Set positions (= runtime ids)
    are preserved; only the membership used by the chooser is filtered.
    Returns an undo callback."""
    from concourse.hw_specs import get_activation_tables
    tables = get_activation_tables("gen3")
    saved = {k: set(v) for k, v in tables.items()}
    keep = "natural_log_exp_and_others"
    for name, fns in tables.items():
        if name != keep:
            fns.discard(EXP)
            fns.discard(LN)

    def undo():
        for k, v in saved.items():
            tables[k].clear()
            tables[k].update(v)
    return undo


def build_nc():
    nc = bacc.Bacc("TRN2", target_bir_lowering=False, debug=False)

    xq = nc.dram_tensor("xq", [P, CT, S], F16, kind="ExternalInput").ap()
    xk = nc.dram_tensor("xk", [P, CT, S], F16, kind="ExternalInput").ap()
    xv = nc.dram_tensor("xv", [P, CT, S], F16, kind="ExternalInput").ap()
    wq = nc.dram_tensor("wq", [P, CT, DC], F16, kind="ExternalInput").ap()
    wk = nc.dram_tensor("wk", [P, CT, DC], F16, kind="ExternalInput").ap()
    wv = nc.dram_tensor("wv", [P, CT, DC], F16, kind="ExternalInput").ap()
    wo = nc.dram_tensor("wo", [HD, HPC, D], F16, kind="ExternalInput").ap()
    bq = nc.dram_tensor("bq", [P, 2], F32, kind="ExternalInput").ap()
    bk = nc.dram_tensor("bk", [P, 2], F32, kind="ExternalInput").ap()
    onec = nc.dram_tensor("onec", [P, HD], F16, kind="ExternalInput").ap()
    outT = nc.dram_tensor("outT", [NOT, P, S], F32, kind="ExternalOutput").ap()

    with tile.TileContext(nc) as tc:
        with tc.tile_pool(name="w", bufs=1) as wpool, \
             tc.tile_pool(name="persist", bufs=1) as persist:
            w_q = wpool.tile([P, CT, DC], F16)
            w_k = wpool.tile([P, CT, DC], F16)
            w_v = wpool.tile([P, CT, DC], F16)
            w_oh = wpool.tile([HD, HPC, D], F16)
            bq_sb = wpool.tile([P, 2], F32)
            bk_sb = wpool.tile([P, 2], F32)
            ident = wpool.tile([P, P], F16)
            ones_sb = wpool.tile([P, HD], F16)

            QT = persist.tile([P, 2, S], F16)    # [p, dtile, i]
            KT = persist.tile([P, 2, S], F16)
            Vp = persist.tile([P, NJT, HPC, HD + 1], F16)  # V' + ones col
            att4 = persist.tile([HD, HPC, S], F16)         # per-head, base 0

            # ---------------- projections ----------------
            with tc.tile_pool(name="xs", bufs=8) as xpool, \
                 tc.tile_pool(name="pp", bufs=8, space="PSUM") as ppool, \
                 tc.tile_pool(name="vt", bufs=4) as vtpool:
                for xin, wsb, wdram, dst, bias in (
                    (xq, w_q, wq, QT, bq_sb),
                    (xk, w_k, wk, KT, bk_sb),
                    (xv, w_v, wv, None, None),
                ):
                    psums = [[ppool.tile([P, ICW], F32, tag="proj",
                                         name=f"ps_{t}_{i}")
                              for i in range(NIC)] for t in range(2)]
                    for ct in range(CT):
                        nc.sync.dma_start(wsb[:, ct, :], wdram[:, ct, :])
                        # split x chunk into halves so matmuls start sooner
                        xh = [xpool.tile([P, S // 2], F16, tag="xstream",
                                         name=f"xh_{ct}_{half}")
                              for half in range(2)]
                        for half in range(2):
                            nc.sync.dma_start(
                                xh[half][:],
                                xin[:, ct,
                                    half * (S // 2):(half + 1) * (S // 2)])
                        if ct == 2 and wdram is wq:
                            # deferred const loads: queue behind the first
                            # chunks so the first matmuls start ASAP
                            nc.sync.dma_start(w_oh[:], wo)
                            nc.sync.dma_start(bq_sb[:], bq)
                            nc.sync.dma_start(bk_sb[:], bk)
                            nc.sync.dma_start(ones_sb[:], onec)
                            make_identity(nc, ident[:])
                            nc.sync.dma_start(
                                Vp[:, :, :, HD:HD + 1],
                                onec.rearrange(
                                    "p (j h) -> p j h", j=NJT)[:, :, :, None])
                        for t in range(2):
                            for ic in range(NIC):
                                half, sub = divmod(ic, 2)
                                nc.tensor.matmul(
                                    psums[t][ic][:, :],
                                    lhsT=wsb[:, ct, t * P:(t + 1) * P],
                                    rhs=xh[half][:, sub * ICW:(sub + 1) * ICW],
                                    start=(ct == 0), stop=(ct == CT - 1),
                                )
                    for t in range(2):
                        for ic in range(NIC):
                            isl = slice(ic * ICW, (ic + 1) * ICW)
                            if dst is not None:
                                nc.vector.tensor_scalar_add(
                                    dst[:, t, isl], psums[t][ic][:, :],
                                    bias[:, t:t + 1],
                                )
                            else:
                                # V^T chunk -> transpose -> V' natural layout
                                vtmp = vtpool.tile([P, ICW], F16, tag="vt")
                                nc.vector.tensor_copy(
                                    vtmp[:], psums[t][ic][:, :])
                                for jj in range(ICW // P):
                                    jt = ic * (ICW // P) + jj
                                    tp = ppool.tile([P, P], F16, tag="proj",
                                                    name=f"tp_{t}_{ic}_{jj}")
                                    nc.tensor.transpose(
                                        tp[:, :],
                                        vtmp[:, jj * P:(jj + 1) * P],
                                        ident[:],
                                    )
                                    nc.vector.tensor_copy(
                                        Vp[:, jt, 2 * t:2 * t + 2, 0:HD],
                                        tp[:, :].rearrange(
                                            "p (h e) -> p h e", h=2),
                                    )

            # ---------------- attention + output projection ----------------
            with tc.tile_pool(name="sc", bufs=2, space="PSUM") as spool, \
                 tc.tile_pool(name="acc", bufs=4, space="PSUM") as apool, \
                 tc.tile_pool(name="pt", bufs=6) as ptpool, \
                 tc.tile_pool(name="nrm", bufs=2) as npool, \
                 tc.tile_pool(name="oev", bufs=4) as oevpool:

                def jt_loop(ic):
                    isl = slice(ic * ICW, (ic + 1) * ICW)
                    accs = [apool.tile([HD + 1, ICW], F32, tag="acc",
                                       name=f"acc_{ic}_{h}")
                            for h in range(HPC)]
                    for jt in range(NJT):
                        for hp in range(2):
                            sc = spool.tile([P, 2 * ICW], F32, tag="sc",
                                            name=f"sc_{ic}_{jt}_{hp}")
                            for hl in range(2):
                                psl = slice(64 * hl, 64 * (hl + 1))
                                nc.tensor.matmul(
                                    sc[:, hl * ICW:(hl + 1) * ICW],
                                    lhsT=KT[psl, hp, jt * P:(jt + 1) * P],
                                    rhs=QT[psl, hp, isl],
                                    start=True, stop=True,
                                )
                            pt = ptpool.tile([P, 2 * ICW], F16, tag="pt",
                                             name=f"pt_{ic}_{jt}_{hp}")
                            nc.scalar.activation(pt[:], sc[:], EXP)
                            for hl in range(2):
                                h = 2 * hp + hl
                                nc.tensor.matmul(
                                    accs[h][:, :],
                                    lhsT=Vp[:, jt, h, :],
                                    rhs=pt[:, hl * ICW:(hl + 1) * ICW],
                                    start=(jt == 0), stop=(jt == NJT - 1),
                                )
                    return accs

                def norm_a(ic, accs):
                    au = npool.tile([HD + 1, HPC, ICW], F32, tag="aun",
                                    name=f"au_{ic}")
                    for h in range(HPC):
                        nc.vector.tensor_copy(au[:, h, :], accs[h][:, :])
                    # 1/sumexp = exp(-ln(sumexp)) on ACT, all heads at once
                    nc.scalar.activation(
                        au[HD:HD + 1, :, :], au[HD:HD + 1, :, :], LN)
                    rec16 = npool.tile([HD + 1, HPC, ICW], F16, tag="rec16",
                                       name=f"rec16_{ic}")
                    nc.scalar.activation(
                        rec16[HD:HD + 1, :, :], au[HD:HD + 1, :, :], EXP,
                        scale=-1.0)
                    return au, rec16

                def norm_b(ic, au, rec16):
                    isl = slice(ic * ICW, (ic + 1) * ICW)
                    for h in range(HPC):
                        bcp = apool.tile([HD, ICW], F32, tag="acc",
                                         name=f"bcp_{ic}_{h}")
                        nc.tensor.matmul(
                            bcp[:, :],
                            lhsT=ones_sb[HD:HD + 1, :],
                            rhs=rec16[HD:HD + 1, h, :],
                            start=True, stop=True,
                        )
                        nc.vector.tensor_mul(
                            att4[:, h, isl], au[0:HD, h, :], bcp[:, :])
                    for ot in range(NOT):
                        po = apool.tile([P, ICW], F32, tag="acc",
                                        name=f"po_{ic}_{ot}")
                        for h in range(HPC):
                            nc.tensor.matmul(
                                po[:, :],
                                lhsT=w_oh[:, h, ot * P:(ot + 1) * P],
                                rhs=att4[:, h, isl],
                                start=(h == 0), stop=(h == HPC - 1),
                            )
                        osb = oevpool.tile([P, ICW], F32, tag="oev",
                                           name=f"osb_{ic}_{ot}")
                        nc.vector.tensor_copy(osb[:], po[:, :])
                        nc.sync.dma_start(outT[ot, :, isl], osb[:])

                pend = None
                for ic in range(NIC):
                    accs = jt_loop(ic)
                    au, rec16 = norm_a(ic, accs)
                    if pend is not None:
                        norm_b(*pend)
                    pend = (ic, au, rec16)
                norm_b(*pend)
    undo = _steer_act_tables()
    try:
        nc.compile()
    finally:
        undo()
    return nc


def get_nc():
    global _CACHED_NC
    if _CACHED_NC is None:
        _CACHED_NC = build_nc()
    return _CACHED_NC


def _prep_x(x):
    # [S, D] -> [P, CT, S] : dev[p, ct, i] = x[i, ct*128+p]
    return np.ascontiguousarray(
        x.reshape(S, CT, P).transpose(2, 1, 0)).astype(np.float16)


def _prep_w(w, scale=1.0):
    # W slice [DC, D] -> [P, CT, DC] : dev[p, ct, d] = W[d, ct*128+p]*scale
    wT = (w.T * scale).astype(np.float32)  # [D, DC]
    return np.ascontiguousarray(
        wT.reshape(CT, P, DC).transpose(1, 0, 2)).astype(np.float16)


def make_in_maps(q, k, v, Wq, bq, Wk, bk, Wv, bv, Wo, bo):
    scale = 1.0 / np.sqrt(HD)
    xs = {}
    for b in range(B):
        xs[b] = (_prep_x(q[b]), _prep_x(k[b]), _prep_x(v[b]))
    in_maps = []
    for core in range(8):
        b, g = divmod(core, 4)
        gs = slice(g * DC, (g + 1) * DC)
        woT = Wo[:, gs].T  # [DC, D]
        in_maps.append({
            "xq": xs[b][0], "xk": xs[b][1], "xv": xs[b][2],
            "wq": _prep_w(Wq[gs], scale),
            "wk": _prep_w(Wk[gs]),
            "wv": _prep_w(Wv[gs]),
            "wo": np.ascontiguousarray(
                woT.reshape(HPC, HD, D).transpose(1, 0, 2)).astype(np.float16),
            "bq": np.ascontiguousarray(
                (bq[gs] * scale).reshape(2, P).T).astype(np.float32),
            "bk": np.ascontiguousarray(
                bk[gs].reshape(2, P).T).astype(np.float32),
            "onec": np.ones((P, HD), np.float16),
        })
    return in_maps


def kernel(q, k, v, Wq, bq, Wk, bk, Wv, bv, Wo, bo, _results_hook=None):
    q = np.asarray(q, np.float32)
    k = np.asarray(k, np.float32)
    v = np.asarray(v, np.float32)
    Wq = np.asarray(Wq, np.float32)
    Wk = np.asarray(Wk, np.float32)
    Wv = np.asarray(Wv, np.float32)
    Wo = np.asarray(Wo, np.float32)
    bq = np.asarray(bq, np.float32)
    bk = np.asarray(bk, np.float32)
    bv = np.asarray(bv, np.float32)
    bo = np.asarray(bo, np.float32)

    nc = get_nc()
    in_maps = make_in_maps(q, k, v, Wq, bq, Wk, bk, Wv, bv, Wo, bo)
    res = run_bass_kernel_spmd(nc, in_maps, core_ids=list(range(8)))
    if _results_hook is not None:
        _results_hook(res)

    const = bo + Wo @ bv  # V-bias folds to a constant through softmax
    out = np.zeros((B, S, D), np.float32)
    for core in range(8):
        b = core // 4
        pT = res.results[core]["outT"]  # [NOT, P, S]
        out[b] += pT.transpose(2, 0, 1).reshape(S, D).astype(np.float32)
    out += const[None, None, :]
    return out
